# revision 4
# baseline (speedup 1.0000x reference)
"""Trainium2 Bass kernel v2: sum-of-7-box-blurs (k=3..15, edge pad) * base_map.

Math: out = base_map * sum_k 1/(7k^2) V_k H_k x. The 2D kernel
KER[v,u] = t(max(|v|,|u|,1)) (15x15) is applied as 15 column-slices:
out[m,n] = sum_u B_u[v] x[m+v, n+u], each B_u a 15-tap vertical band.

PE: fp8e4 DoubleRow matmuls pack 2 column-slices per pass (8 passes
cover u=-7..7 + one zero slot; ~1 cyc/col measured) with NO DVE delta
materialization — vs 15 single-slice passes otherwise. The step%16
constraint on the DoubleRow k-tile pair is met by storing x
column-DEINTERLEAVED mod 16 (18 phase-planes of 272 B, planes 16/17
duplicate 0/1 shifted one element): a column shift becomes a plane
offset (step 272|16).

Precision: x is quantized to fp8e4m3 with row-wise error diffusion
(quant noise -> high frequency, killed by the box sums); band weights
are fp8 with per-column error diffusion, scale 256 folded into the
PSUM evacuation. Measured end-to-end rel err ~7e-3 (gate 2e-2).

Output path: ScalarE evacuates PSUM (scale 1/256 -> fp16), DVE
multiplies by fp16 base_map (2x mode), fp16 store; host re-interleaves
columns and casts to fp32. Host-side work is index shuffling + dtype
conversion only; all FLOPs (conv + mask multiply) run on device.

Sharding: rows split across 8 cores; halos come from host edge-padding
so cores are fully independent. DMA: all x/base_map tiles prefetch
up front on the sync HWDGE queue (wait-free instructions round-robin
across all 16 DMA engines; dependency-gated ones pin to engine 64),
and stores go through the gpsimd SWDGE, which spreads regardless.
"""

import numpy as np
import ml_dtypes

import concourse.bass as bass
import concourse.mybir as mybir
import concourse.tile as tile
from concourse import bacc, bass_utils

H = W = 4096
NC = 8
RPC = H // NC                 # 512 output rows per core
PAD = 7
PR = RPC + 2 * PAD            # 526 padded rows per core
ROW_TILES = [(0, 114), (114, 114), (228, 114), (342, 114), (456, 56)]
K_SIZES = [3, 5, 7, 9, 11, 13, 15]
WS = 256.0                    # weight scale folded into PSUM evacuation

NPL = 18                      # phase planes (16 + 2 shifted duplicates)
PLANE = 272                   # bytes per plane (257 used, 16-aligned)
XROW = NPL * PLANE            # 4896 B per row of deinterleaved x
NJ = 256                      # j-extent of one matmul (output cols / 16)
PAIRS = [(0, 1), (2, 3), (4, 5), (6, 7), (8, 9), (10, 11), (12, 13), (14, 15)]

F8 = mybir.dt.float8e4
F16 = mybir.dt.float16
F32 = mybir.dt.float32
NP_F8 = ml_dtypes.float8_e4m3fn


def _ker2d() -> np.ndarray:
    """Exact 15x15 kernel: sum_k 1/(7k^2) box_k ⊗ box_k."""
    ker = np.zeros((15, 15))
    for k in K_SIZES:
        p = k // 2
        ker[PAD - p:PAD + p + 1, PAD - p:PAD + p + 1] += 1.0 / (k * k * len(K_SIZES))
    return ker


def _quant_weights() -> np.ndarray:
    """Column-bands quantized to fp8 grid (x WS) with per-column error
    diffusion along v so each column's total mass is preserved."""
    ker = _ker2d()
    wq = np.zeros_like(ker)
    for u in range(15):
        r = 0.0
        for v in range(15):
            t = ker[v, u] + r
            q = float(np.float32(t * WS).astype(NP_F8))
            wq[v, u] = q          # scaled value, exactly on fp8 grid
            r = t - q / WS
    return wq


def _weights_np() -> np.ndarray:
    """lhsT [8, 128, 2, 128] fp8: lhsT[p, k, t, m] = Wq[k-m-7, u_t]."""
    wq = _quant_weights()
    lhst = np.zeros((8, 128, 2, 128), dtype=np.float32)
    for pi, pair in enumerate(PAIRS):
        for t, dd in enumerate(pair):
            if dd >= 15:
                continue          # zero slot
            u = dd - 7
            for m in range(128):
                k0, k1 = m, min(m + 15, 128)
                lhst[pi, k0:k1, t, m] = wq[0:k1 - k0, u + 7]
    return lhst.astype(NP_F8)


def _rowdiffuse_fp8(a: np.ndarray) -> np.ndarray:
    """Quantize fp32->fp8e4m3 with 1D error diffusion along each row."""
    out = np.empty(a.shape, dtype=NP_F8)
    err = np.zeros(a.shape[0], np.float32)
    for j in range(a.shape[1]):
        t = a[:, j] + err
        q = t.astype(NP_F8)
        out[:, j] = q
        err = t - q.astype(np.float32)
    return out


def _col_perm() -> np.ndarray:
    """P[d] = image column held at device column d (d = 2048h+256qq+j)."""
    d = np.arange(W)
    h, r = d // 2048, d % 2048
    qq, j = r // NJ, r % NJ
    return 16 * j + 8 * h + qq


def _kernel_body(nc, tc, xd_d, bm_d, w_d, out_d):
    mult = mybir.AluOpType.mult
    copy_f = mybir.ActivationFunctionType.Copy

    with (
        tc.tile_pool(name="wpool", bufs=1) as wpool,
        tc.tile_pool(name="xpool", bufs=5) as xpool,
        tc.tile_pool(name="bmpool", bufs=5) as bmpool,
        tc.tile_pool(name="ppool", bufs=3) as ppool,
        tc.tile_pool(name="opool", bufs=3) as opool,
        tc.tile_pool(name="psum", bufs=2, space="PSUM") as psum_pool,
    ):
        wsb = wpool.tile([128, 8 * 256], F8)
        nc.sync.dma_start(
            out=wsb.rearrange("k (j t m) -> k j t m", j=8, t=2),
            in_=w_d.rearrange("j k t m -> k j t m"))

        # PE warmup during the initial DMA fill (HAM un-throttle).
        warm = psum_pool.tile([128, 2048], F32, tag="ps", name="warm")
        for i in range(24):
            nc.tensor.matmul(
                warm[:, (i % 4) * 512:(i % 4 + 1) * 512],
                wsb[:, 0:128], wsb[:, 0:512], start=(i < 4), stop=(i >= 20))

        def chunks(n, k):
            cuts = [(i * n) // k for i in range(k + 1)]
            return list(zip(cuts[:-1], cuts[1:]))

        # Prefetch every row-tile's x and base_map up front: the queue's
        # throughput scales with outstanding DMA instructions (engines join
        # per-instruction), and all 5 tiles fit in SBUF.
        def load_tile(rt, Mt, nx=1):
            Krows = min(128, PR - rt)
            x_sb = xpool.tile([128, XROW], F8, tag="x")
            for r0, r1 in chunks(Krows, nx):
                nc.sync.dma_start(out=x_sb[r0:r1], in_=xd_d[rt + r0:rt + r1])
            bm_sb = bmpool.tile([128, W], F16, tag="bm")
            nc.sync.dma_start(out=bm_sb[:Mt], in_=bm_d[rt:rt + Mt])
            return x_sb, bm_sb

        loaded = [load_tile(*rt, nx=(4 if i == 0 else 1))
                  for i, rt in enumerate(ROW_TILES)]
        for ri, (rt, Mt) in enumerate(ROW_TILES):
            Krows = min(128, PR - rt)
            x_sb, bm_sb = loaded[ri]
            xd4 = x_sb.rearrange("k (pl e) -> k pl e", pl=NPL)
            osb = opool.tile([128, W], F16, tag="o")

            for half in (0, 1):
                ps = psum_pool.tile([128, 2048], F32, tag="ps")
                # start=True marks pending-zero per PSUM BANK: exactly one
                # start matmul per 512-col bank (odd qq first), pass 0's
                # even-qq matmuls re-issued at the end with start=False.
                sched = ([(0, [1, 3, 5, 7])]
                         + [(pi, list(range(8))) for pi in range(1, 8)]
                         + [(0, [0, 2, 4, 6])])
                for si, (pi, qqs) in enumerate(sched):
                    da, db = PAIRS[pi]
                    lhsT = wsb[:Krows, pi * 256:(pi + 1) * 256].rearrange(
                        "k (t m) -> k t m", t=2)
                    for ei, qq in enumerate(qqs):
                        q = 8 * half + qq
                        if q + db <= NPL - 1:
                            pa, jo = q + da, 0
                        else:
                            pa, jo = q + da - 16, 1
                        nc.tensor.matmul(
                            ps[:, qq * NJ:(qq + 1) * NJ],
                            lhsT,
                            xd4[:Krows, pa:pa + 2, jo:jo + NJ],
                            start=(si == 0), stop=(si == len(sched) - 1),
                            perf_mode=mybir.MatmulPerfMode.DoubleRow)

                psc = ppool.tile([128, 2048], F16, tag="psc")
                nc.scalar.activation(
                    out=psc[:Mt], in_=ps[:Mt], func=copy_f, scale=1.0 / WS)
                nc.vector.tensor_tensor(
                    out=osb[:Mt, half * 2048:(half + 1) * 2048],
                    in0=psc[:Mt],
                    in1=bm_sb[:Mt, half * 2048:(half + 1) * 2048], op=mult)
                # stores through the gpsimd SOFTWARE DGE: the hardware DGE
                # pins dependency-gated instructions to one DMA engine,
                # while SWDGE descriptors spread across all 16 engines
                c0 = half * 2048
                for r0, r1 in chunks(Mt, 3):
                    nc.gpsimd.dma_start(
                        out=out_d[rt + r0:rt + r1, c0:c0 + 2048],
                        in_=osb[r0:r1, c0:c0 + 2048])


def _dedup_ldweights(nc):
    """bass legalization emits one InstLdweights per matmul. The PE reloads
    256 weight columns (~213ns) each time, dominating the DR matmul stream
    (~60-120ns). Weights are identical within each (half, pass) group, so
    drop consecutive duplicate loads from the final scheduled stream,
    migrating any dependency edges onto the next PE instruction."""
    removed = {}
    for blk in nc.m.functions[0].blocks:
        new = []
        last_sig = None
        stash = []
        for inst in blk.instructions:
            nm = type(inst).__name__
            if nm == "InstLdweights":
                w = inst.ins[0]
                sig = (w.memref, w.offset, str(w.ap), str(inst.perf_mode),
                       str(inst.is_transpose))
                if sig == last_sig and not inst.has_wait():
                    stash.append(inst)
                    continue
                last_sig = sig
            elif nm == "InstMatmult":
                if inst.is_transpose:
                    last_sig = None
            elif inst.engine == mybir.EngineType.PE:
                last_sig = None
            if stash and getattr(inst, "engine", None) == mybir.EngineType.PE:
                for s in stash:
                    inst.merge_dependencies_from(s)
                    removed[s.name] = inst.name
                stash = []
            new.append(inst)
        assert not stash
        blk.instructions = new
    if removed:
        for blk in nc.m.functions[0].blocks:
            for inst in blk.instructions:
                inst.remap_dependency_names(removed)
    n = sum(1 for b in nc.m.functions[0].blocks for i in b.instructions
            if type(i).__name__ == "InstLdweights")
    print(f"ldweights dedup: removed {len(removed)}, kept {n}")


def _build():
    nc = bacc.Bacc("TRN2", target_bir_lowering=False, debug=False)
    xd_d = nc.dram_tensor("xd", [PR, XROW], F8, kind="ExternalInput").ap()
    bm_d = nc.dram_tensor("bm", [RPC, W], F16, kind="ExternalInput").ap()
    w_d = nc.dram_tensor("wts", [8, 128, 2, 128], F8, kind="ExternalInput").ap()
    out_d = nc.dram_tensor("out", [RPC, W], F16, kind="ExternalOutput").ap()
    with tile.TileContext(nc) as tc:
        _kernel_body(nc, tc, xd_d, bm_d, w_d, out_d)
    nc.compile()
    _dedup_ldweights(nc)
    return nc


_CACHE: dict = {}


def _get_nc():
    if "nc" not in _CACHE:
        _CACHE["nc"] = _build()
    return _CACHE["nc"]


def _in_maps(x: np.ndarray, base_map: np.ndarray) -> list[dict]:
    xp = np.pad(x, PAD, mode="edge")
    xp = np.pad(xp, ((0, 0), (0, 4)), mode="edge")      # [4110, 4114]
    xq = _rowdiffuse_fp8(xp)

    # deinterleave: plane p elem e holds column 16e+p (e 0..256 used)
    e_idx = np.arange(257)
    cols = 16 * e_idx[None, :] + np.arange(NPL)[:, None]  # [18, 257]
    dint = np.zeros((xq.shape[0], NPL, PLANE), dtype=NP_F8)
    dint[:, :, :257] = xq[:, cols]
    dint = dint.reshape(xq.shape[0], XROW)

    perm = _col_perm()
    bmd = base_map[:, perm].astype(np.float16)
    wts = _weights_np()

    maps = []
    for c in range(NC):
        maps.append({
            "xd": np.ascontiguousarray(dint[c * RPC: c * RPC + PR]),
            "bm": np.ascontiguousarray(bmd[c * RPC:(c + 1) * RPC]),
            "wts": wts,
        })
    return maps


def run(x, base_map, **kwargs) -> tuple[np.ndarray, bass_utils.BassKernelResults]:
    x = np.ascontiguousarray(np.asarray(x), dtype=np.float32)
    base_map = np.ascontiguousarray(np.asarray(base_map), dtype=np.float32)
    nc = _get_nc()
    res = bass_utils.run_bass_kernel_spmd(
        nc, _in_maps(x, base_map), core_ids=list(range(NC)), **kwargs)
    dev = np.concatenate([np.asarray(r["out"]) for r in res.results], axis=0)
    out = np.empty((H, W), dtype=np.float32)
    out[:, _col_perm()] = dev.astype(np.float32)
    return out[None, None], res


def kernel(x, base_map) -> np.ndarray:
    return run(x, base_map)[0]
